# revision 30
# baseline (speedup 1.0000x reference)
"""Distance-correlation (DisCo) loss kernel for Trainium2, sharded over 8 NeuronCores.

Math: reference computes NxN pairwise |vi-vj| matrices (a, b), weighted row
means, double centering, then scalar reductions.  Everything except the
genuinely 2-D term

    Q_ab[i] = sum_j w_j * |v1_i - v1_j| * |v2_i - v2_j|

has an exact O(N log N) closed form on the host (sorted prefix sums for
weighted |.| row sums, polynomial identities for squared terms).  The device
computes Q_ab only, with rows i sharded across the 8 cores (1024 rows/core,
interleaved: core c owns global rows r with r % 8 == c).

Device kernel (w == 1 fast path, `sym2`): M_ij = |v1_i-v1_j|*|v2_i-v2_j| is
symmetric, so only the block upper triangle is computed.  Per core, i-block b
covers the core's 128 rows of global 1024-row band b.  A custom fused DVE op
computes |v1_j-v1_i|*|v2_j-v2_i| and its free-dim row sum in one pass; for
off-diagonal blocks a ones-vector TensorE matmul (f32r, full rate)
accumulates the partition-dim column sums in PSUM.  The replicated j-side
source rows are fp16 (inputs are fp16-quantized consistently on host and
device; rel. effect on the final scalar ~4e-4), packed [v1 band | v2 band]
per 1024-band and broadcast-DMA'd in descending processing order so the
first compute op only waits for the first (single-band) DMA.  Bands are
processed in descending block count so the TensorE column-sum chains and
their PSUM->SBUF copies finish early; the 8 diagonal bands (row sums only)
run last, hiding the column-sum output DMA.
"""

import functools
import os

import numpy as np

N = 8192
CORES = 8
ROWS = N // CORES          # 1024 rows per core
NIB = ROWS // 128          # 8 partition blocks per core
BD = 1024                  # band width
NB = N // BD               # 8 bands == NIB
PROC = [7, 6, 5, 4, 3, 2, 1, 0]          # band processing order
DMA_GROUPS = [[7], [6, 5], [4, 3], [2, 1], [0]]  # broadcast DMA batching

LAST_RESULT = None         # BassKernelResults of the most recent launch


@functools.lru_cache(maxsize=1)
def _disco_op():
    """Fused DVE op: out = |in0-s0| * |in1-s1|, accum_out = sum(out).

    Registered at runtime into concourse.dve_ops; the uop table ships in
    the NEFF, so no firmware support is needed.  Exactly fills the 8-stage
    v3 DVE pipeline (2 subs, 2 negates, 2 maxes, 1 mul, 1 accum-add).
    """
    from operator import add

    import concourse.dve_ops as D
    from concourse.dve_spec import Spec, Src0, Src1, C0, C1, Zero, maxx, lower
    from concourse.dve_uop import DveOpSpec

    d1 = Src0 - C0
    d2 = Src1 - C1
    body = maxx(d1, Zero - d1) * maxx(d2, Zero - d2)

    def ref(in0, in1, s0, s1, imm2):
        b = (
            np.abs(in0.astype(np.float32) - s0) * np.abs(in1.astype(np.float32) - s1)
        ).astype(np.float32)
        return b, b.reshape(b.shape[0], -1).sum(axis=-1, keepdims=True)

    spec = Spec(body=body, accum=add, accum_init=Zero, reference=ref)
    name = "DISCO_ABSPROD_REDUCE"
    row = max(D._SUB_OPCODE_FOR_NAME.values()) + 1
    D._SUB_OPCODE_FOR_NAME[name] = row
    sha3 = DveOpSpec(
        name=name, opcode=row, uops=lower(spec, ver="v3"), rd1_en=True
    ).sha("v3")
    op = D.DveOp(name, spec, subdim=False, uops_sha={"v3": sha3})
    D.OPS.append(op)
    D.CUSTOM_DVE_SPECS[name] = spec
    return op


@functools.lru_cache(maxsize=3)
def _build(mode: str):
    """mode: 'sym2' (w==1 fast path) or 'weighted' (general w)."""
    if mode == "sym2":
        return _build_sym2()
    import concourse.bacc as bacc
    import concourse.bass as bass
    import concourse.tile as tile
    from concourse import mybir

    f32 = mybir.dt.float32
    nc = bacc.Bacc("TRN2", target_bir_lowering=False, debug=False)

    JC = 2048
    NJC = N // JC
    BCH = 1024

    v1d = nc.dram_tensor("v1", [N], f32, kind="ExternalInput")
    v2d = nc.dram_tensor("v2", [N], f32, kind="ExternalInput")
    wd = nc.dram_tensor("w", [N], f32, kind="ExternalInput")
    # vipack columns: [vi1 | -vi1 | vi2 | -vi2], each NIB wide, partition-major.
    vipackd = nc.dram_tensor("vipack", [128, 4 * NIB], f32, kind="ExternalInput")
    qabd = nc.dram_tensor("qab", [128, NIB], f32, kind="ExternalOutput")

    def bcast(ap1d):
        return bass.AP(
            tensor=ap1d.tensor, offset=ap1d.offset, ap=[[0, 128]] + list(ap1d.ap)
        )

    sub = mybir.AluOpType.subtract
    mult = mybir.AluOpType.mult
    add = mybir.AluOpType.add

    with tile.TileContext(nc) as tc:
        with (
            tc.tile_pool(name="singles", bufs=1) as singles,
            tc.tile_pool(name="ab", bufs=2) as pab,
            tc.tile_pool(name="scrap", bufs=1) as pscrap,
        ):
            v1rep = singles.tile([128, N], f32)
            v2rep = singles.tile([128, N], f32)
            wrep = singles.tile([128, N], f32)
            reps = [(v1rep, v1d), (v2rep, v2d), (wrep, wd)]
            for c in range(N // BCH):
                for rep, src in reps:
                    sap = src.ap()
                    nc.sync.dma_start(
                        out=rep[:, c * BCH : (c + 1) * BCH],
                        in_=bcast(sap[c * BCH : (c + 1) * BCH]),
                    )

            vipack = singles.tile([128, 4 * NIB], f32)
            nc.sync.dma_start(out=vipack[:, :], in_=vipackd.ap())
            vi1 = vipack[:, 0 * NIB : 1 * NIB]
            nvi1 = vipack[:, 1 * NIB : 2 * NIB]
            vi2 = vipack[:, 2 * NIB : 3 * NIB]
            nvi2 = vipack[:, 3 * NIB : 4 * NIB]

            qacc = singles.tile([128, NIB], f32)
            for ib in range(NIB):
                for jc in range(NJC):
                    j0 = jc * JC
                    ab = pab.tile([128, 2, JC], f32, tag="ab")
                    a = ab[:, 0, :]
                    b = ab[:, 1, :]
                    for t, (rep, vis, nvis) in enumerate(
                        ((v1rep, vi1, nvi1), (v2rep, vi2, nvi2))
                    ):
                        nc.scalar.activation(
                            out=ab[:, t, :],
                            in_=rep[:, j0 : j0 + JC],
                            func=mybir.ActivationFunctionType.Abs,
                            bias=nvis[:, ib : ib + 1],
                            scale=1.0,
                        )
                    wb = pab.tile([128, JC], f32, tag="wb")
                    nc.vector.tensor_tensor(wb, b, wrep[:, j0 : j0 + JC], mult)
                    scrap = pscrap.tile([128, JC], f32)
                    nc.vector.tensor_tensor(scrap, a, wb, mult)
                    # in-place copy whose op1 performs the free-dim
                    # reduction, chained across j chunks via scalar2 init
                    nc.vector.tensor_scalar(
                        scrap,
                        scrap,
                        1.0,
                        (0.0 if jc == 0 else qacc[:, ib : ib + 1]),
                        mult,
                        add,
                        accum_out=qacc[:, ib : ib + 1],
                    )

            nc.sync.dma_start(out=qabd.ap(), in_=qacc[:, :])

    nc.compile()
    return nc


# modeled per-op engine costs (ns) used for greedy channel load balancing
_C_DVE_FUSED = 1131
_C_DVE_PREP = 330       # fp16 tensor_scalar v-s (signed), 4x mode
_C_ACT_PREP = 995       # Copy activation with bias=-s (signed diff)
_C_POOL_PREP = 853      # gpsimd tensor_scalar v-s (signed)
_C_POOL_PROD = 853      # gpsimd tensor_tensor product (signed, fp16)
_C_ACT_RED = 995        # Abs activation + accum_out: |prod| tile + row sums
_C_ACT_COL = 1038       # PSUM -> SBUF column-sum copy
_C_DVE_COL = 1255
# ACT fixed costs paid before compute: act-table load + warmup op + vipack DMA
_C_ACT_FIXED = 2778


def _build_sym2(reps: int = 1):
    """Symmetric fast path (w == 1), fp16 sources, see module docstring.

    Each (band, scalar-block) pair is dispatched to one of two compute
    channels, chosen by a greedy modeled-load balancer: the fused custom
    DVE op, or a signed-diff channel exploiting |x*y| = |x|*|y|: two
    SIGNED (v_j - v_i) fp16 prep tiles built on the least-loaded of
    DVE/ACT/GpSimd (tensor_scalar subtract / Copy activation with
    negative bias -- no abs needed), a gpsimd tensor_tensor multiply,
    and one ACT Abs activation whose accum_out port row-reduces while
    its main output IS the |M| block tile the column-sum matmul needs.
    This keeps DVE, ACT and Pool all busy instead of serializing
    everything through DVE.
    """
    import concourse.bacc as bacc
    import concourse.bass as bass
    import concourse.tile as tile
    from concourse import mybir

    f32 = mybir.dt.float32
    f16 = mybir.dt.float16
    f32r = mybir.dt.float32r
    mult = mybir.AluOpType.mult
    sub = mybir.AluOpType.subtract
    amax = mybir.AluOpType.abs_max

    nc = bacc.Bacc("TRN2", target_bir_lowering=False, debug=False)
    # vjpack rows are bands in PROC order; each row = [v1 band | v2 band] fp16.
    vjpackd = nc.dram_tensor("vjpack", [NB, 2 * BD], f16, kind="ExternalInput")
    # vipack columns: [vi1 | vi2 | -vi1 | -vi2], each NIB wide, partition-major.
    vipackd = nc.dram_tensor("vipack", [128, 4 * NIB], f32, kind="ExternalInput")
    qrowd = nc.dram_tensor("qrow", [128, NIB, NB], f32, kind="ExternalOutput")
    qcold = nc.dram_tensor("qcol", [NB - 1, BD], f32, kind="ExternalOutput")

    def bcast(ap2d):
        return bass.AP(
            tensor=ap2d.tensor, offset=ap2d.offset, ap=[[0, 128]] + list(ap2d.ap)
        )

    op = _disco_op()
    with tile.TileContext(nc) as tc:
        with (
            tc.tile_pool(name="singles", bufs=1) as singles,
            tc.tile_pool(name="scrap", bufs=12) as pscrap,
            tc.tile_pool(name="ab", bufs=10) as pab,
            tc.tile_pool(name="psum", bufs=4, space="PSUM") as ppsum,
        ):
            # the tiny scalar-pack DMA gates every compute op -- issue it on
            # the ACT HWDGE ring so it runs parallel to the first broadcast
            vipack = singles.tile([128, 4 * NIB], f32)
            nc.scalar.dma_start(out=vipack[:, :], in_=vipackd.ap())
            vi1 = vipack[:, 0 * NIB : 1 * NIB]
            vi2 = vipack[:, 1 * NIB : 2 * NIB]
            nvi1 = vipack[:, 2 * NIB : 3 * NIB]
            nvi2 = vipack[:, 3 * NIB : 4 * NIB]

            # broadcast the fp16 [v1 | v2] band rows, in processing order,
            # batched so later groups ride behind the first compute ops
            vband = {}
            for g, bands in enumerate(DMA_GROUPS):
                t = singles.tile([128, len(bands) * 2 * BD], f16, tag=f"vb{g}")
                r0 = PROC.index(bands[0])
                sap = vjpackd.ap()  # [NB, 2*BD]
                nc.sync.dma_start(
                    out=t[:, :],
                    in_=bcast(sap[r0 : r0 + len(bands)]),
                )
                for k, b in enumerate(bands):
                    vband[b] = (
                        t[:, k * 2 * BD : k * 2 * BD + BD],
                        t[:, k * 2 * BD + BD : (k + 1) * 2 * BD],
                    )

            qacc = singles.tile([128, NIB, NB], f32)
            nc.vector.memset(qacc, 0.0)
            ones_f = singles.tile([128, 1], f32)
            nc.vector.memset(ones_f, 1.0)
            ones = singles.tile([128, 1], f32r)
            nc.vector.tensor_copy(ones[:, :], ones_f[:, :])
            ones16 = singles.tile([128, 1], f16)
            nc.vector.tensor_copy(ones16[:, :], ones_f[:, :])
            # flat column-sum staging tile on partition 0; one output DMA
            colbuf = singles.tile([1, (NB - 1) * BD], f32)
            # dummy scalar op: pull the ACT table load off the critical
            # path (overlaps the broadcast DMAs)
            nc.scalar.activation(
                out=colbuf[:, 0:1],
                in_=ones_f[0:1, 0:1],
                func=mybir.ActivationFunctionType.Abs,
                bias=0.0,
                scale=1.0,
            )

            # static plan from minimising modeled max engine load:
            # 13 signed-diff channel blocks spread over the 36-block
            # sequence, preps mostly on Pool, column sums on ACT
            ch_slots = {1, 4, 7, 9, 12, 15, 18, 20, 23, 26, 29, 31, 33}
            prep_engs = iter(
                ["P", "P", "P", "A", "P", "P", "P", "D", "P", "P", "P", "A",
                 "P", "P", "P", "D", "P", "P", "P", "A", "P", "A", "P", "D",
                 "P", "P"]
            )
            seqno = [0]

            def emit_prep(t, jc, b, ab):
                """Build signed (v_t - s) into ab[:, t, :], planned engine."""
                vc = vband[jc][t]
                vis = (vi1, vi2)[t]
                nvis = (nvi1, nvi2)[t]
                eng = next(prep_engs, "P")
                if eng == "A":
                    # Abs (AP bias is only legal for non-Copy funcs); the
                    # channel tolerates abs'd or signed diffs equally
                    nc.scalar.activation(
                        out=ab[:, t, :],
                        in_=vc,
                        func=mybir.ActivationFunctionType.Abs,
                        bias=nvis[:, b : b + 1],
                        scale=1.0,
                    )
                else:
                    e = nc.vector if eng == "D" else nc.gpsimd
                    e.tensor_scalar(
                        ab[:, t, :], vc, vis[:, b : b + 1], None, sub
                    )

            def choose_channel(force_fused=False):
                """True -> signed-diff channel, False -> fused DVE."""
                s = seqno[0]
                seqno[0] += 1
                return (not force_fused) and s in ch_slots

            def emit_fused(jc, b):
                scrap = pscrap.tile([128, BD], f32r, tag="scrapr")
                nc.vector._custom_dve(
                    op,
                    out=scrap[:, :],
                    in0=vband[jc][0],
                    in1=vband[jc][1],
                    s0=vi1[:, b : b + 1],
                    s1=vi2[:, b : b + 1],
                    accum_out=qacc[:, b, jc : jc + 1],
                )
                return scrap

            def emit_product(ab):
                prod = pab.tile([128, BD], f16, tag="prod")
                nc.gpsimd.tensor_tensor(
                    prod[:, :], ab[:, 0, :], ab[:, 1, :], mult
                )
                return prod

            def emit_reduce(jc, b, prod):
                scrap = pscrap.tile([128, BD], f16, tag="scrap16")
                nc.scalar.activation(
                    out=scrap[:, :],
                    in_=prod[:, :],
                    func=mybir.ActivationFunctionType.Abs,
                    bias=0.0,
                    scale=1.0,
                    accum_out=qacc[:, b, jc : jc + 1],
                )
                return scrap

            def emit_colsum(jc, pt):
                dst = colbuf[:, (jc - 1) * BD : jc * BD]
                # GPSIMD cannot access PSUM; the plan puts these on ACT
                nc.scalar.copy(dst, pt[:, :])

            for _ in range(reps):
                # Bands in processing order; the 8 diagonals form a final
                # pseudo-band (no TensorE work).  Channel preps for band
                # i+1 are emitted during band i (one-band software
                # pipeline) so Pool products and ACT reduces never wait
                # on same-band preps.
                bands = []
                for jc in range(NB - 1, 0, -1):
                    blocks = [(jc, b) for b in range(jc)]
                    bands.append((jc, blocks, True))
                bands.append((None, [(jc, jc) for jc in range(NB - 1, -1, -1)],
                              False))
                plans = []
                for _, blocks, _ in bands:
                    plans.append([choose_channel(force_fused=(jc <= 1 and b == jc))
                                  for jc, b in blocks])

                def emit_band_preps(i):
                    abts = {}
                    for (jc, b), ch in zip(bands[i][1], plans[i]):
                        if ch:
                            abt = pab.tile([128, 2, BD], f16, tag="ab")
                            abts[(jc, b)] = abt
                            emit_prep(0, jc, b, abt)
                            emit_prep(1, jc, b, abt)
                    return abts

                abts = emit_band_preps(0)
                for i, (band_jc, blocks, has_mm) in enumerate(bands):
                    chs = plans[i]
                    # products (preps landed a band ago)
                    prods = {}
                    for (jc, b), ch in zip(blocks, chs):
                        if ch:
                            prods[(jc, b)] = emit_product(abts[(jc, b)])
                    if has_mm:
                        pt = ppsum.tile([1, BD], f32, tag="pt")
                    else:
                        pt = None
                    nmm = [0, 0]
                    total_mm = len(blocks)

                    def mm(scrap, o):
                        for h in range(BD // 512):
                            nc.tensor.matmul(
                                pt[:, h * 512 : (h + 1) * 512],
                                o[:, :],
                                scrap[:, h * 512 : (h + 1) * 512],
                                start=(nmm[h] == 0),
                                stop=(nmm[h] == total_mm - 1),
                            )
                            nmm[h] += 1

                    # fused DVE blocks
                    for (jc, b), ch in zip(blocks, chs):
                        if not ch:
                            scrap = emit_fused(jc, b)
                            if has_mm:
                                mm(scrap, ones)
                    if not has_mm:
                        # all fused diagonals (incl. row 0's) are done;
                        # ship row 0 while the channel reduces finish
                        nc.sync.dma_start(
                            out=qrowd.ap()[:, 0:1, :], in_=qacc[:, 0:1, :]
                        )
                    # next band's preps ride behind this band's compute
                    next_abts = (emit_band_preps(i + 1)
                                 if i + 1 < len(bands) else {})
                    # reduces (+ their matmuls)
                    for (jc, b), ch in zip(blocks, chs):
                        if ch:
                            scrap = emit_reduce(jc, b, prods[(jc, b)])
                            if has_mm:
                                mm(scrap, ones16)
                    if has_mm:
                        emit_colsum(band_jc, pt)
                        if band_jc == 1:
                            nc.sync.dma_start(
                                out=qcold.ap(), in_=colbuf[:, :]
                            )
                    else:
                        nc.sync.dma_start(
                            out=qrowd.ap()[:, 1:, :], in_=qacc[:, 1:, :]
                        )
                    abts = next_abts

    nc.compile()
    return nc


def _abs_weighted_sums(q, x):
    """out_i = sum_j q_j * |x_i - x_j|, exact via sorting (float64)."""
    o = np.argsort(x, kind="stable")
    xs, qs = x[o], q[o]
    cq = np.cumsum(qs)
    cqx = np.cumsum(qs * xs)
    vals = xs * (2.0 * cq - cq[-1]) + cqx[-1] - 2.0 * cqx
    out = np.empty_like(vals)
    out[o] = vals
    return out


class _CachedRunner:
    """One-time-jitted SPMD executor (same lowering as bass2jax
    run_bass_via_pjrt, but the jitted callable is retained so repeat
    kernel() calls skip retracing/recompilation)."""

    def __init__(self, nc, n_cores=CORES):
        import jax
        from jax.experimental.shard_map import shard_map
        from jax.sharding import Mesh, PartitionSpec

        import concourse.mybir as mybir
        from concourse.bass2jax import (
            _bass_exec_p,
            install_neuronx_cc_hook,
            partition_id_tensor,
        )

        install_neuronx_cc_hook()
        self.n_cores = n_cores
        part_name = nc.partition_id_tensor.name if nc.partition_id_tensor else None
        in_names, out_names, out_avals, zero_outs = [], [], [], []
        for alloc in nc.m.functions[0].allocations:
            if not isinstance(alloc, mybir.MemoryLocationSet):
                continue
            name = alloc.memorylocations[0].name
            if alloc.kind == "ExternalInput":
                if name != part_name:
                    in_names.append(name)
            elif alloc.kind == "ExternalOutput":
                out_names.append(name)
                shape = tuple(alloc.tensor_shape)
                dtype = mybir.dt.np(alloc.dtype)
                out_avals.append(jax.core.ShapedArray(shape, dtype))
                zero_outs.append(np.zeros(shape, dtype))
        self.in_names, self.out_names = in_names, out_names
        self.zero_outs = zero_outs
        n_params = len(in_names)
        all_names = in_names + out_names
        if part_name is not None:
            all_names = all_names + [part_name]

        def _body(*args):
            operands = list(args)
            if part_name is not None:
                operands.append(partition_id_tensor())
            return tuple(
                _bass_exec_p.bind(
                    *operands,
                    out_avals=tuple(out_avals),
                    in_names=tuple(all_names),
                    out_names=tuple(out_names),
                    lowering_input_output_aliases=(),
                    sim_require_finite=True,
                    sim_require_nnan=True,
                    nc=nc,
                )
            )

        devices = jax.devices()[:n_cores]
        mesh = Mesh(np.asarray(devices), ("core",))
        nin = n_params + len(out_names)
        self.fn = jax.jit(
            shard_map(
                _body,
                mesh=mesh,
                in_specs=(PartitionSpec("core"),) * nin,
                out_specs=(PartitionSpec("core"),) * len(out_names),
                check_rep=False,
            ),
            donate_argnums=tuple(range(n_params, nin)),
            keep_unused=True,
        )

    def run(self, in_maps):
        n = self.n_cores
        concat_in = [
            np.concatenate([np.asarray(in_maps[c][k]) for c in range(n)], axis=0)
            for k in self.in_names
        ]
        concat_zero = [np.concatenate([z] * n, axis=0) for z in self.zero_outs]
        outs = [np.asarray(o) for o in self.fn(*concat_in, *concat_zero)]
        per_core = []
        for c in range(n):
            d = {}
            for k, o in zip(self.out_names, outs):
                m = o.shape[0] // n
                d[k] = o[c * m : (c + 1) * m]
            per_core.append(d)
        return per_core


_RUNNER_CACHE = {}


def _make_in_map(v1, v2, w, mode, c):
    """v1/v2 are fp32 (already fp16-quantized values in sym2 mode)."""
    if mode == "sym2":
        rows = v1[c::8]
        rows2 = v2[c::8]
        vr1 = np.ascontiguousarray(rows).reshape(NIB, 128).T
        vr2 = np.ascontiguousarray(rows2).reshape(NIB, 128).T
        vband = v1.reshape(NB, BD)
        wband = v2.reshape(NB, BD)
        vj = np.concatenate([vband, wband], axis=1)[PROC]  # [NB, 2*BD]
        return {
            "vjpack": np.ascontiguousarray(vj.astype(np.float16)),
            "vipack": np.ascontiguousarray(
                np.concatenate([vr1, vr2, -vr1, -vr2], axis=1)
            ),
        }
    rows = v1[c * ROWS : (c + 1) * ROWS]
    rows2 = v2[c * ROWS : (c + 1) * ROWS]
    vr1 = np.ascontiguousarray(rows).reshape(NIB, 128).T
    vr2 = np.ascontiguousarray(rows2).reshape(NIB, 128).T
    return {
        "v1": v1,
        "v2": v2,
        "w": w,
        "vipack": np.ascontiguousarray(
            np.concatenate([vr1, -vr1, vr2, -vr2], axis=1)
        ),
    }


def _run_device_qab(v1, v2, w, ones):
    global LAST_RESULT
    mode = os.environ.get("DISCO_MODE") or ("sym2" if ones else "weighted")
    nc = _build(mode)
    trace = os.environ.get("DISCO_TRACE", "0") == "1"
    in_maps = [_make_in_map(v1, v2, w, mode, c) for c in range(CORES)]
    if trace or os.environ.get("DISCO_NO_RUNNER_CACHE", "0") == "1":
        from concourse.bass_utils import run_bass_kernel_spmd

        res = run_bass_kernel_spmd(
            nc, in_maps, core_ids=list(range(CORES)), trace=trace
        )
        LAST_RESULT = res
        results = res.results
    else:
        runner = _RUNNER_CACHE.get(mode)
        if runner is None:
            runner = _CachedRunner(nc)
            _RUNNER_CACHE[mode] = runner
        results = runner.run(in_maps)

    if mode == "sym2":
        qab = np.empty(N, dtype=np.float64)
        colsum = np.zeros((NB - 1, BD), dtype=np.float64)
        for c, r in enumerate(results):
            qab[c::8] = r["qrow"].astype(np.float64).sum(axis=2).T.reshape(ROWS)
            colsum += r["qcol"].astype(np.float64)
        for band in range(1, NB):
            qab[band * BD : (band + 1) * BD] += colsum[band - 1]
        return qab
    parts = []
    for r in results:
        q = r["qab"].astype(np.float64)
        parts.append(q.T.reshape(ROWS))  # [p, ib] -> row ib*128+p
    return np.concatenate(parts)


def kernel(var_1, var_2, normedweight, power):
    v1 = np.ascontiguousarray(np.asarray(var_1, dtype=np.float32))
    v2 = np.ascontiguousarray(np.asarray(var_2, dtype=np.float32))
    w = np.ascontiguousarray(np.asarray(normedweight, dtype=np.float32))
    p = int(np.asarray(power))
    ones = bool(np.all(w == np.float32(1.0)))

    if ones:
        # quantize consistently: device j-side tiles are fp16, so use the
        # same quantized values for the i-side scalars and host corrections
        v1 = v1.astype(np.float16).astype(np.float32)
        v2 = v2.astype(np.float16).astype(np.float32)

    qab = _run_device_qab(v1, v2, w, ones)

    v1d, v2d, wd = v1.astype(np.float64), v2.astype(np.float64), w.astype(np.float64)
    u = _abs_weighted_sums(wd, v1d) / N
    v = _abs_weighted_sums(wd, v2d) / N
    W = wd.sum()
    ga = (wd * u).mean()
    gb = (wd * v).mean()
    al = u - ga
    be = v - gb
    Qaa = W * v1d**2 - 2.0 * v1d * (wd * v1d).sum() + (wd * v1d**2).sum()
    Qbb = W * v2d**2 - 2.0 * v2d * (wd * v2d).sum() + (wd * v2d**2).sum()
    Duu = (wd * u * u).sum()
    Duv = (wd * u * v).sum()
    Dvv = (wd * v * v).sum()
    Rawu = _abs_weighted_sums(wd * u, v1d)
    Rawv = _abs_weighted_sums(wd * v, v1d)
    Rbwu = _abs_weighted_sums(wd * u, v2d)
    Rbwv = _abs_weighted_sums(wd * v, v2d)

    k = 2.0 * N - W
    SAA = Qaa - 2.0 * Rawu + Duu - al**2 * k
    SBB = Qbb - 2.0 * Rbwv + Dvv - be**2 * k
    SAB = qab - Rawv - Rbwu + Duv - al * be * k

    num = (np.abs(SAB) / N * wd).mean()
    denA = (SAA / N * wd).mean()
    denB = (SBB / N * wd).mean()
    EPS = 1e-12
    with np.errstate(all="ignore"):
        if p == 1:
            d = np.abs(denA * denB)
            out = num / np.sqrt(d + EPS)
        elif p == 2:
            d = np.abs(denA * denB)
            out = num**2 / (d + EPS)
        else:
            out = (num / np.sqrt(denA * denB) + EPS) ** p
    if np.isnan(out):
        out = 0.0
    out = max(out, 0.0)
    return np.float32(out)
